# revision 33
# baseline (speedup 1.0000x reference)
"""Bass/Tile TRN2 kernel for nn_Attention (additive/Bahdanau-style attention).

reference math per batch b:
  res_q = query[b] @ W_q.T                      (Q, H)
  res_c = context[b] @ W_c.T + b_c              (C, H)
  logit[q,c] = sum_h W_o[h]*tanh(res_c[c,h] + res_q[q,h]) + b_o
  w = mask * exp(logit); weights = w / (sum_c w + eps)
  out = weights @ context[b]

Sharding: data-parallel over batch B=8 across the 8 NeuronCores (one batch
per core). The big (Q,C,H) intermediate is never materialized in HBM: tanh
tiles live in SBUF and are immediately contracted against W_o on the PE.

Layout: H on partitions for the tanh stage, so res_q[q,:]+b_c is a
per-partition ACT bias and one ACT instruction computes tanh(res_cT + bias)
for a whole (128, C) tile. The W_o contraction uses the tanh tile as the
matmul stationary operand, producing logitT columns [c_chunk(128), 1] —
full-partition PSUM writes (PE can only address PSUM at partition offsets
{0,32,64}). The whole softmax then runs in transposed [c, q] layout, which
is exactly the lhsT the final weights@context matmul needs, and the c-sum
is a ones-vector matmul. Host-side transposes of the inputs remove all
on-device input transposition; the wtsT output is un-transposed on host.
"""

import numpy as np

B, Q, C, D, H = 8, 64, 512, 512, 256
EPS = 1e-5
P = 128
KD = D // P   # 4 chunks of the contraction dim d
KC = C // P   # 4 chunks of the context dim c
JH = H // P   # 2 chunks of the hidden dim h
N_CORES = 8


def _build_program(b_o_val: float):
    import concourse.bacc as bacc
    import concourse.mybir as mybir
    import concourse.tile as tile
    from contextlib import ExitStack

    F32 = mybir.dt.float32
    BF16 = mybir.dt.bfloat16
    Act = mybir.ActivationFunctionType

    nc = bacc.Bacc("TRN2", target_bir_lowering=False, debug=False)

    F32R = mybir.dt.float32r
    qT_d = nc.dram_tensor("qT", [D, Q], F32R, kind="ExternalInput")
    ctx_d = nc.dram_tensor("ctx", [C, D], F32R, kind="ExternalInput")
    ctxT_d = nc.dram_tensor("ctxT", [D, C], F32R, kind="ExternalInput")
    maskB_d = nc.dram_tensor("maskB", [P, KC, Q], F32, kind="ExternalInput")
    WqT_d = nc.dram_tensor("WqT", [D, H], F32R, kind="ExternalInput")
    WcT_d = nc.dram_tensor("WcT", [D, H], F32R, kind="ExternalInput")
    Wo2_d = nc.dram_tensor("Wo2", [P, JH], BF16, kind="ExternalInput")
    bc2_d = nc.dram_tensor("bc2", [P, JH], F32, kind="ExternalInput")
    out_d = nc.dram_tensor("out", [Q, D], F32, kind="ExternalOutput")
    wtsT_d = nc.dram_tensor("wtsT", [C, Q], F32, kind="ExternalOutput")

    with tile.TileContext(nc) as tc, ExitStack() as ctx:
        const = ctx.enter_context(tc.tile_pool(name="const", bufs=1))
        tmp_pool = ctx.enter_context(tc.tile_pool(name="tmp", bufs=6))
        sm_pool = ctx.enter_context(tc.tile_pool(name="softmax", bufs=1))
        ps_small = ctx.enter_context(
            tc.tile_pool(name="ps_small", bufs=3, space="PSUM")
        )
        ps_rc = ctx.enter_context(tc.tile_pool(name="ps_rc", bufs=3, space="PSUM"))
        ps_lt = ctx.enter_context(tc.tile_pool(name="ps_lt", bufs=1, space="PSUM"))

        # ---- input loads; DMA triggers serialize on the sequencer, so the
        # critical-path tensors (W_cT/ctxT for res_c, W_qT/qT for the bias)
        # go first and the tail-only tensors are issued after the main loop.
        ctxT_sb = const.tile([P, KD, C], F32R)
        ctxT_ap = ctxT_d.ap().rearrange("(k p) c -> p k c", p=P)
        nc.sync.dma_start(ctxT_sb[:, 0:2, :], ctxT_ap[:, 0:2, :])
        nc.sync.dma_start(ctxT_sb[:, 2:4, :], ctxT_ap[:, 2:4, :])
        WcT_sb = const.tile([P, KD, H], F32R)
        nc.sync.dma_start(WcT_sb[:], WcT_d.ap().rearrange("(k p) h -> p k h", p=P))
        WqT_sb = const.tile([P, KD, H], F32R)
        nc.sync.dma_start(WqT_sb[:], WqT_d.ap().rearrange("(k p) h -> p k h", p=P))
        qT_sb = const.tile([P, KD, Q], F32R)
        nc.sync.dma_start(qT_sb[:], qT_d.ap().rearrange("(k p) q -> p k q", p=P))
        bc_sb = const.tile([P, JH], F32)
        nc.gpsimd.dma_start(bc_sb[:], bc2_d.ap())
        Wo_sb = const.tile([P, JH], BF16)
        nc.gpsimd.dma_start(Wo_sb[:], Wo2_d.ap())
        ctx_sb = const.tile([P, KC, D], F32R)
        maskB_sb = const.tile([P, KC, Q], F32)

        # ---- per h-chunk: res_cT -> SBUF (bf16, so the broadcast-adds run in
        # DVE 4x mode) and bias[h, q] = res_qT[h, q] + b_c[h]
        bias_sb = const.tile([P, JH, Q], F32)
        rc_sb = const.tile([P, JH, C], BF16)
        for j in range(JH):
            prc = ps_rc.tile([P, C], F32)
            for k in range(KD):
                nc.tensor.matmul(
                    prc[:],
                    WcT_sb[:, k, j * P : (j + 1) * P],
                    ctxT_sb[:, k, :],
                    start=(k == 0),
                    stop=(k == KD - 1),
                )
            prq = ps_small.tile([P, Q], F32, tag="small")
            for k in range(KD):
                nc.tensor.matmul(
                    prq[:],
                    WqT_sb[:, k, j * P : (j + 1) * P],
                    qT_sb[:, k, :],
                    start=(k == 0),
                    stop=(k == KD - 1),
                )
            nc.vector.tensor_copy(rc_sb[:, j, :], prc[:])
            nc.vector.tensor_scalar_add(bias_sb[:, j, :], prq[:], bc_sb[:, j : j + 1])

        # ---- main loop over blocks of G queries:
        #  DVE: s[h, g, c] = res_cT[h, c] + bias[h, q]   (tensor_scalar, 4x bf16)
        #  ACT: one big bias-free tanh per (block, j) -> bf16
        #  PE:  per-q W_o contraction with the tanh tile as stationary,
        #       producing logitT columns [c_chunk, 1] (PE can only write PSUM
        #       at partition offsets {0,32,64})
        # The softmax + output runs per q-half (its own PSUM logit tile) so
        # the first half hides inside the main loop; the ACT table set
        # (exp_and_others) holds both Tanh and Exp, so no mid-loop reloads.
        HQ = Q // 2
        bo_sb = sm_pool.tile([P, 1], F32)
        nc.vector.memset(bo_sb[:], float(b_o_val))
        ones_col = sm_pool.tile([P, 1], F32)
        nc.vector.memset(ones_col[:], 1.0)
        ones_row = sm_pool.tile([1, P], F32)
        nc.vector.memset(ones_row[:], 1.0)
        out_sb = sm_pool.tile([Q, D], F32)
        wT_sb = sm_pool.tile([P, KC, Q], F32)
        lt_ps = [
            ps_lt.tile([P, KC, HQ], F32, name=f"lth{h}", tag=f"lth{h}")
            for h in range(2)
        ]

        def emit_block(q0, G):
            tmps = []
            for j in range(JH):
                s = tmp_pool.tile([P, 16, C], BF16, name="s", tag="s", bufs=2)
                for g in range(G):
                    q = q0 + g
                    nc.vector.tensor_scalar_add(
                        s[:, g, :], rc_sb[:, j, :], bias_sb[:, j, q : q + 1]
                    )
                t = tmp_pool.tile([P, 16, C], BF16, name="t", tag="t", bufs=3)
                nc.scalar.activation(t[:, :G, :], s[:, :G, :], Act.Tanh)
                tmps.append(t)
            for g in range(G):
                q = q0 + g
                lt = lt_ps[q // HQ]
                for k in range(KC):
                    for j in range(JH):
                        nc.tensor.matmul(
                            lt[:, k, (q % HQ) : (q % HQ) + 1],
                            tmps[j][:, g, k * P : (k + 1) * P],
                            Wo_sb[:, j : j + 1],
                            start=(j == 0),
                            stop=(j == JH - 1),
                        )

        def emit_exp(h):
            expT = sm_pool.tile([P, KC, HQ], F32, name=f"expT{h}", tag=f"expT{h}")
            nc.scalar.activation(expT[:], lt_ps[h][:], Act.Exp, bias=bo_sb[:, 0:1])
            return expT

        def emit_mask_ou(h, expT):
            qlo = h * HQ
            wexpT = sm_pool.tile(
                [P, KC, HQ], F32R, name=f"wexpT{h}", tag=f"wexpT{h}"
            )
            nc.vector.tensor_mul(
                wexpT[:], expT[:], maskB_sb[:, :, qlo : qlo + HQ]
            )
            wexpT_f = wexpT.bitcast(F32)
            sq_ps = ps_small.tile([HQ, 1], F32, name=f"sq{h}", tag="small")
            for k in range(KC):
                nc.tensor.matmul(
                    sq_ps[:],
                    wexpT_f[:, k, :],
                    ones_col[:],
                    start=(k == 0),
                    stop=(k == KC - 1),
                )
            # un-normalized output; normalization applied after the sums
            ou_ps = ps_rc.tile([HQ, D], F32, name=f"ou{h}", tag="prc")
            for k in range(KC):
                nc.tensor.matmul(
                    ou_ps[:],
                    wexpT[:, k, :],
                    ctx_sb[:, k, :],
                    start=(k == 0),
                    stop=(k == KC - 1),
                )
            return wexpT, sq_ps, ou_ps

        def emit_norm(h, sq_ps, ou_ps):
            qlo = h * HQ
            recipQ = sm_pool.tile([HQ, 1], F32, name=f"recipQ{h}", tag=f"recipQ{h}")
            nc.vector.tensor_scalar_add(recipQ[:], sq_ps[:], float(EPS))
            nc.vector.reciprocal(recipQ[:], recipQ[:])
            nc.vector.tensor_scalar_mul(
                out_sb[qlo : qlo + HQ, :], ou_ps[:], recipQ[:, 0:1]
            )
            nc.sync.dma_start(out_d.ap()[qlo : qlo + HQ, :], out_sb[qlo : qlo + HQ, :])

        def emit_wts_half(h, wexpT):
            # weights output in [c, q] layout; pure side chain
            qlo = h * HQ
            wexpT_f = wexpT.bitcast(F32)
            s_ps = ps_small.tile([1, HQ], F32, name=f"sr{h}", tag="small")
            for k in range(KC):
                nc.tensor.matmul(
                    s_ps[:],
                    ones_col[:],
                    wexpT_f[:, k, :],
                    start=(k == 0),
                    stop=(k == KC - 1),
                )
            recip = sm_pool.tile([1, HQ], F32, name=f"recip{h}", tag=f"recip{h}")
            nc.vector.tensor_scalar_add(recip[:], s_ps[:], float(EPS))
            nc.vector.reciprocal(recip[:], recip[:])
            rb_ps = ps_rc.tile([P, HQ], F32, name=f"rb{h}", tag="prc")
            nc.tensor.matmul(rb_ps[:], ones_row[:], recip[:], start=True, stop=True)
            for k in range(KC):
                nc.vector.tensor_mul(
                    wT_sb[:, k, qlo : qlo + HQ], wexpT_f[:, k, :], rb_ps[:]
                )
            nc.sync.dma_start(
                wtsT_d.ap().rearrange("(k p) q -> p k q", p=P)[:, :, qlo : qlo + HQ],
                wT_sb[:, :, qlo : qlo + HQ],
            )

        q0 = 0
        for G in [4, 12]:
            emit_block(q0, G)
            q0 += G
        # markers: the tail-only DMAs WAW-depend on these, so the big ctx
        # transfer does not contend with the critical-path input DMAs
        nc.vector.tensor_copy(ctx_sb[0:1, 0, 0:1], bias_sb[0:1, 0, 0:1].bitcast(F32R))
        nc.vector.tensor_copy(maskB_sb[0:1, 0, 0:1], bias_sb[0:1, 0, 0:1])
        nc.sync.dma_start(ctx_sb[:], ctx_d.ap().rearrange("(k p) d -> p k d", p=P))
        nc.sync.dma_start(maskB_sb[:], maskB_d.ap())
        emit_block(q0, 16)
        q0 += 16
        emit_block(q0, 16)
        q0 += 16
        expT0 = emit_exp(0)
        emit_block(q0, 14)
        q0 += 14
        emit_block(q0, 2)
        q0 += 2
        t0_parts = emit_mask_ou(0, expT0)
        emit_norm(0, t0_parts[1], t0_parts[2])
        expT1 = emit_exp(1)
        t1_parts = emit_mask_ou(1, expT1)
        emit_norm(1, t1_parts[1], t1_parts[2])
        emit_wts_half(0, t0_parts[0])
        emit_wts_half(1, t1_parts[0])

    nc.compile()
    return nc


def make_in_maps(query, context, mask, W_c, b_c, W_q, W_o):
    f32 = np.float32
    import ml_dtypes

    WqT = np.ascontiguousarray(np.asarray(W_q, f32).T)  # (D, H)
    WcT = np.ascontiguousarray(np.asarray(W_c, f32).T)  # (D, H)
    Wo2 = np.ascontiguousarray(
        np.asarray(W_o, f32).reshape(JH, P).T.astype(ml_dtypes.bfloat16)
    )  # (P, JH) bf16
    bc2 = np.ascontiguousarray(np.asarray(b_c, f32).reshape(JH, P).T)  # (P, JH)
    in_maps = []
    for b in range(B):
        in_maps.append(
            {
                "qT": np.ascontiguousarray(np.asarray(query[b], f32).T),
                "ctx": np.ascontiguousarray(np.asarray(context[b], f32)),
                "ctxT": np.ascontiguousarray(np.asarray(context[b], f32).T),
                "maskB": np.ascontiguousarray(
                    np.broadcast_to(
                        np.asarray(mask[b], f32).reshape(KC, P).T[:, :, None],
                        (P, KC, Q),
                    )
                ),
                "WqT": WqT,
                "WcT": WcT,
                "Wo2": Wo2,
                "bc2": bc2,
            }
        )
    return in_maps


def kernel(query, context, mask, W_c, b_c, W_q, W_o, b_o):
    from concourse.bass_utils import run_bass_kernel_spmd

    nc = _build_program(float(np.asarray(b_o)))
    in_maps = make_in_maps(query, context, mask, W_c, b_c, W_q, W_o)
    res = run_bass_kernel_spmd(nc, in_maps, list(range(N_CORES))).results
    out = np.stack([res[b]["out"] for b in range(B)])
    wts = np.stack([np.ascontiguousarray(res[b]["wtsT"].T) for b in range(B)])
    return out, wts


# revision 34
# speedup vs baseline: 1.0375x; 1.0375x over previous
"""Bass/Tile TRN2 kernel for nn_Attention (additive/Bahdanau-style attention).

reference math per batch b:
  res_q = query[b] @ W_q.T                      (Q, H)
  res_c = context[b] @ W_c.T + b_c              (C, H)
  logit[q,c] = sum_h W_o[h]*tanh(res_c[c,h] + res_q[q,h]) + b_o
  w = mask * exp(logit); weights = w / (sum_c w + eps)
  out = weights @ context[b]

Sharding: data-parallel over batch B=8 across the 8 NeuronCores (one batch
per core). The big (Q,C,H) intermediate is never materialized in HBM: tanh
tiles live in SBUF and are immediately contracted against W_o on the PE.

Layout: H on partitions for the tanh stage, so res_q[q,:]+b_c is a
per-partition ACT bias and one ACT instruction computes tanh(res_cT + bias)
for a whole (128, C) tile. The W_o contraction uses the tanh tile as the
matmul stationary operand, producing logitT columns [c_chunk(128), 1] —
full-partition PSUM writes (PE can only address PSUM at partition offsets
{0,32,64}). The whole softmax then runs in transposed [c, q] layout, which
is exactly the lhsT the final weights@context matmul needs, and the c-sum
is a ones-vector matmul. Host-side transposes of the inputs remove all
on-device input transposition; the wtsT output is un-transposed on host.
"""

import numpy as np

B, Q, C, D, H = 8, 64, 512, 512, 256
EPS = 1e-5
P = 128
KD = D // P   # 4 chunks of the contraction dim d
KC = C // P   # 4 chunks of the context dim c
JH = H // P   # 2 chunks of the hidden dim h
N_CORES = 8


def _build_program(b_o_val: float):
    import concourse.bacc as bacc
    import concourse.mybir as mybir
    import concourse.tile as tile
    from contextlib import ExitStack

    F32 = mybir.dt.float32
    BF16 = mybir.dt.bfloat16
    Act = mybir.ActivationFunctionType

    nc = bacc.Bacc("TRN2", target_bir_lowering=False, debug=False)

    F32R = mybir.dt.float32r
    qT_d = nc.dram_tensor("qT", [D, Q], F32R, kind="ExternalInput")
    ctx_d = nc.dram_tensor("ctx", [C, D], F32R, kind="ExternalInput")
    ctxT_d = nc.dram_tensor("ctxT", [D, C], F32R, kind="ExternalInput")
    maskB_d = nc.dram_tensor("maskB", [P, KC, Q], F32, kind="ExternalInput")
    WqT_d = nc.dram_tensor("WqT", [D, H], F32R, kind="ExternalInput")
    WcT_d = nc.dram_tensor("WcT", [D, H], F32R, kind="ExternalInput")
    Wo2_d = nc.dram_tensor("Wo2", [P, JH], BF16, kind="ExternalInput")
    bc2_d = nc.dram_tensor("bc2", [P, JH], F32, kind="ExternalInput")
    out_d = nc.dram_tensor("out", [Q, D], F32, kind="ExternalOutput")
    wtsT_d = nc.dram_tensor("wtsT", [C, Q], F32, kind="ExternalOutput")

    with tile.TileContext(nc) as tc, ExitStack() as ctx:
        const = ctx.enter_context(tc.tile_pool(name="const", bufs=1))
        tmp_pool = ctx.enter_context(tc.tile_pool(name="tmp", bufs=6))
        sm_pool = ctx.enter_context(tc.tile_pool(name="softmax", bufs=1))
        ps_small = ctx.enter_context(
            tc.tile_pool(name="ps_small", bufs=3, space="PSUM")
        )
        ps_rc = ctx.enter_context(tc.tile_pool(name="ps_rc", bufs=3, space="PSUM"))
        ps_lt = ctx.enter_context(tc.tile_pool(name="ps_lt", bufs=1, space="PSUM"))

        # ---- input loads; DMA triggers serialize on the sequencer, so the
        # critical-path tensors (W_cT/ctxT for res_c, W_qT/qT for the bias)
        # go first and the tail-only tensors are issued after the main loop.
        ctxT_sb = const.tile([P, KD, C], F32R)
        ctxT_ap = ctxT_d.ap().rearrange("(k p) c -> p k c", p=P)
        nc.sync.dma_start(ctxT_sb[:, 0:2, :], ctxT_ap[:, 0:2, :])
        nc.sync.dma_start(ctxT_sb[:, 2:4, :], ctxT_ap[:, 2:4, :])
        WcT_sb = const.tile([P, KD, H], F32R)
        nc.sync.dma_start(WcT_sb[:], WcT_d.ap().rearrange("(k p) h -> p k h", p=P))
        WqT_sb = const.tile([P, KD, H], F32R)
        nc.sync.dma_start(WqT_sb[:], WqT_d.ap().rearrange("(k p) h -> p k h", p=P))
        qT_sb = const.tile([P, KD, Q], F32R)
        nc.sync.dma_start(qT_sb[:], qT_d.ap().rearrange("(k p) q -> p k q", p=P))
        bc_sb = const.tile([P, JH], F32)
        nc.gpsimd.dma_start(bc_sb[:], bc2_d.ap())
        Wo_sb = const.tile([P, JH], BF16)
        nc.gpsimd.dma_start(Wo_sb[:], Wo2_d.ap())
        ctx_sb = const.tile([P, KC, D], F32R)
        maskB_sb = const.tile([P, KC, Q], F32)

        # ---- per h-chunk: res_cT -> SBUF (bf16, so the broadcast-adds run in
        # DVE 4x mode) and bias[h, q] = res_qT[h, q] + b_c[h]
        bias_sb = const.tile([P, JH, Q], F32)
        rc_sb = const.tile([P, JH, C], BF16)
        for j in range(JH):
            prc = ps_rc.tile([P, C], F32)
            for k in range(KD):
                nc.tensor.matmul(
                    prc[:],
                    WcT_sb[:, k, j * P : (j + 1) * P],
                    ctxT_sb[:, k, :],
                    start=(k == 0),
                    stop=(k == KD - 1),
                )
            prq = ps_small.tile([P, Q], F32, tag="small")
            for k in range(KD):
                nc.tensor.matmul(
                    prq[:],
                    WqT_sb[:, k, j * P : (j + 1) * P],
                    qT_sb[:, k, :],
                    start=(k == 0),
                    stop=(k == KD - 1),
                )
            nc.vector.tensor_copy(rc_sb[:, j, :], prc[:])
            nc.vector.tensor_scalar_add(bias_sb[:, j, :], prq[:], bc_sb[:, j : j + 1])

        # ---- main loop over blocks of G queries:
        #  DVE: s[h, g, c] = res_cT[h, c] + bias[h, q]   (tensor_scalar, 4x bf16)
        #  ACT: one big bias-free tanh per (block, j) -> bf16
        #  PE:  per-q W_o contraction with the tanh tile as stationary,
        #       producing logitT columns [c_chunk, 1] (PE can only write PSUM
        #       at partition offsets {0,32,64})
        # The softmax + output runs per q-half (its own PSUM logit tile) so
        # the first half hides inside the main loop; the ACT table set
        # (exp_and_others) holds both Tanh and Exp, so no mid-loop reloads.
        HQ = Q // 2
        bo_sb = sm_pool.tile([P, 1], F32)
        nc.vector.memset(bo_sb[:], float(b_o_val))
        ones_col = sm_pool.tile([P, 1], F32)
        nc.vector.memset(ones_col[:], 1.0)
        ones_row = sm_pool.tile([1, P], F32)
        nc.vector.memset(ones_row[:], 1.0)
        out_sb = sm_pool.tile([Q, D], F32)
        wT_sb = sm_pool.tile([P, KC, Q], F32)
        lt_ps = [
            ps_lt.tile([P, KC, HQ], F32, name=f"lth{h}", tag=f"lth{h}")
            for h in range(2)
        ]

        def emit_block(q0, G):
            tmps = []
            for j in range(JH):
                s = tmp_pool.tile([P, 16, C], BF16, name="s", tag="s", bufs=2)
                for g in range(G):
                    q = q0 + g
                    nc.vector.tensor_scalar_add(
                        s[:, g, :], rc_sb[:, j, :], bias_sb[:, j, q : q + 1]
                    )
                t = tmp_pool.tile([P, 16, C], BF16, name="t", tag="t", bufs=3)
                nc.scalar.activation(t[:, :G, :], s[:, :G, :], Act.Tanh)
                tmps.append(t)
            for g in range(G):
                q = q0 + g
                lt = lt_ps[q // HQ]
                for k in range(KC):
                    for j in range(JH):
                        nc.tensor.matmul(
                            lt[:, k, (q % HQ) : (q % HQ) + 1],
                            tmps[j][:, g, k * P : (k + 1) * P],
                            Wo_sb[:, j : j + 1],
                            start=(j == 0),
                            stop=(j == JH - 1),
                        )

        def emit_exp(h):
            expT = sm_pool.tile([P, KC, HQ], F32, name=f"expT{h}", tag=f"expT{h}")
            nc.scalar.activation(expT[:], lt_ps[h][:], Act.Exp, bias=bo_sb[:, 0:1])
            return expT

        def emit_mask_ou(h, expT):
            qlo = h * HQ
            wexpT = sm_pool.tile(
                [P, KC, HQ], F32R, name=f"wexpT{h}", tag=f"wexpT{h}"
            )
            nc.vector.tensor_mul(
                wexpT[:], expT[:], maskB_sb[:, :, qlo : qlo + HQ]
            )
            wexpT_f = wexpT.bitcast(F32)
            sq_ps = ps_small.tile([HQ, 1], F32, name=f"sq{h}", tag="small")
            for k in range(KC):
                nc.tensor.matmul(
                    sq_ps[:],
                    wexpT_f[:, k, :],
                    ones_col[:],
                    start=(k == 0),
                    stop=(k == KC - 1),
                )
            # un-normalized output; normalization applied after the sums
            ou_ps = ps_rc.tile([HQ, D], F32, name=f"ou{h}", tag="prc")
            for k in range(KC):
                nc.tensor.matmul(
                    ou_ps[:],
                    wexpT[:, k, :],
                    ctx_sb[:, k, :],
                    start=(k == 0),
                    stop=(k == KC - 1),
                )
            return wexpT, sq_ps, ou_ps

        def emit_norm(h, sq_ps, ou_ps):
            qlo = h * HQ
            recipQ = sm_pool.tile([HQ, 1], F32, name=f"recipQ{h}", tag=f"recipQ{h}")
            nc.vector.tensor_scalar_add(recipQ[:], sq_ps[:], float(EPS))
            nc.vector.reciprocal(recipQ[:], recipQ[:])
            nc.vector.tensor_scalar_mul(
                out_sb[qlo : qlo + HQ, :], ou_ps[:], recipQ[:, 0:1]
            )
            nc.sync.dma_start(out_d.ap()[qlo : qlo + HQ, :], out_sb[qlo : qlo + HQ, :])

        def emit_wts_half(h, wexpT):
            # weights output in [c, q] layout; pure side chain
            qlo = h * HQ
            wexpT_f = wexpT.bitcast(F32)
            s_ps = ps_small.tile([1, HQ], F32, name=f"sr{h}", tag="small")
            for k in range(KC):
                nc.tensor.matmul(
                    s_ps[:],
                    ones_col[:],
                    wexpT_f[:, k, :],
                    start=(k == 0),
                    stop=(k == KC - 1),
                )
            recip = sm_pool.tile([1, HQ], F32, name=f"recip{h}", tag=f"recip{h}")
            nc.vector.tensor_scalar_add(recip[:], s_ps[:], float(EPS))
            nc.vector.reciprocal(recip[:], recip[:])
            rb_ps = ps_rc.tile([P, HQ], F32, name=f"rb{h}", tag="prc")
            nc.tensor.matmul(rb_ps[:], ones_row[:], recip[:], start=True, stop=True)
            for k in range(KC):
                nc.vector.tensor_mul(
                    wT_sb[:, k, qlo : qlo + HQ], wexpT_f[:, k, :], rb_ps[:]
                )
            nc.sync.dma_start(
                wtsT_d.ap().rearrange("(k p) q -> p k q", p=P)[:, :, qlo : qlo + HQ],
                wT_sb[:, :, qlo : qlo + HQ],
            )

        q0 = 0
        for G in [4, 12]:
            emit_block(q0, G)
            q0 += G
        # markers: the tail-only DMAs WAW-depend on these, so the big ctx
        # transfer does not contend with the critical-path input DMAs
        nc.vector.memset(ctx_sb[0:1, 0, 0:1].bitcast(F32), 0.0)
        nc.vector.memset(maskB_sb[0:1, 0, 0:1], 0.0)
        nc.sync.dma_start(ctx_sb[:], ctx_d.ap().rearrange("(k p) d -> p k d", p=P))
        nc.sync.dma_start(maskB_sb[:], maskB_d.ap())
        emit_block(q0, 16)
        q0 += 16
        emit_block(q0, 16)
        q0 += 16
        expT0 = emit_exp(0)
        emit_block(q0, 12)
        q0 += 12
        t0_parts = emit_mask_ou(0, expT0)
        emit_block(q0, 4)
        q0 += 4
        emit_norm(0, t0_parts[1], t0_parts[2])
        expT1 = emit_exp(1)
        t1_parts = emit_mask_ou(1, expT1)
        emit_norm(1, t1_parts[1], t1_parts[2])
        emit_wts_half(0, t0_parts[0])
        emit_wts_half(1, t1_parts[0])

    nc.compile()
    return nc


def make_in_maps(query, context, mask, W_c, b_c, W_q, W_o):
    f32 = np.float32
    import ml_dtypes

    WqT = np.ascontiguousarray(np.asarray(W_q, f32).T)  # (D, H)
    WcT = np.ascontiguousarray(np.asarray(W_c, f32).T)  # (D, H)
    Wo2 = np.ascontiguousarray(
        np.asarray(W_o, f32).reshape(JH, P).T.astype(ml_dtypes.bfloat16)
    )  # (P, JH) bf16
    bc2 = np.ascontiguousarray(np.asarray(b_c, f32).reshape(JH, P).T)  # (P, JH)
    in_maps = []
    for b in range(B):
        in_maps.append(
            {
                "qT": np.ascontiguousarray(np.asarray(query[b], f32).T),
                "ctx": np.ascontiguousarray(np.asarray(context[b], f32)),
                "ctxT": np.ascontiguousarray(np.asarray(context[b], f32).T),
                "maskB": np.ascontiguousarray(
                    np.broadcast_to(
                        np.asarray(mask[b], f32).reshape(KC, P).T[:, :, None],
                        (P, KC, Q),
                    )
                ),
                "WqT": WqT,
                "WcT": WcT,
                "Wo2": Wo2,
                "bc2": bc2,
            }
        )
    return in_maps


def kernel(query, context, mask, W_c, b_c, W_q, W_o, b_o):
    from concourse.bass_utils import run_bass_kernel_spmd

    nc = _build_program(float(np.asarray(b_o)))
    in_maps = make_in_maps(query, context, mask, W_c, b_c, W_q, W_o)
    res = run_bass_kernel_spmd(nc, in_maps, list(range(N_CORES))).results
    out = np.stack([res[b]["out"] for b in range(B)])
    wts = np.stack([np.ascontiguousarray(res[b]["wtsT"].T) for b in range(B)])
    return out, wts
